# revision 5
# baseline (speedup 1.0000x reference)
"""Pairwise Euclidean distance kernel for Trainium2 (8 NeuronCores, SPMD). v3

Math/sharding as v2 (symmetry: each core computes the 5 circulant
column-chunks d=(J-I) mod 8 in {0..4} of its 1024-row block, host mirrors
the other 24 blocks; host-side sq prep; f16 device output upcast on host).

Microbench-driven restructuring (measured on this silicon):
  - K=2 rank-1 matmuls run ~4x slow (~495 ns vs ~125 ns for a chained
    K=128 512-col f16 matmul) and poison surrounding accumulation chains
    (~3.2 us/chunk). The -0.5*sq_j fold is instead a third K=128 matmul:
    sq hi/lo rows zero-padded to 128 partitions (sqp), all-ones rows 0,1
    stationary (w01).
  - Switching the PE stationary inside an accumulation chain costs ~106 ns
    per matmul; consecutive same-stationary matmuls run at ~125 ns. So
    matmuls are emitted K-MAJOR: for a fixed row-tile r, sweep each
    stationary (lhs0, lhs1, w01) across up to 8 PSUM subtiles before
    switching (3 weight switches per 4-chunk group instead of 24).
  - The device outputs d2 = -2*psum + sq_i (affine only, f16); the sqrt
    runs on the host. This turns the epilogue into a dtype-converting
    affine that either ACT or DVE can do, split 3:2 across them (neither
    engine alone can keep up with the PE).
  - sqp (126 zero rows) and w01 are built by on-device memsets + a tiny
    [2, NCOL] sq DMA instead of shipping 1.3 MB of zeros.

The diagonal is exactly zero mathematically; the device still min-clamps
the i==j block so d2 >= 0 there, and the host zeroes the diagonal.

Error vs f32 reference ~1.5e-4 relative (f16 input rounding + f16 d2
rounding; the old diagonal-cancellation term is gone with the host zero).
"""

import sys

try:
    import concourse.bass as _probe  # noqa: F401
except ImportError:
    sys.path.insert(0, "/opt/trn_rl_repo")

import numpy as np

import concourse.bacc as bacc
import concourse.mybir as mybir
from concourse import tile
from concourse.bass_utils import run_bass_kernel_spmd

N = 8192          # number of points
D = 256           # feature dim
NCORES = 8
RPC = N // NCORES  # 1024 rows per core
RT = RPC // 128    # 8 row-tiles per core
JCHUNK = 1024      # output chunk width (2 PSUM banks)
NJC = 5            # circulant offsets d=0..4 (symmetry: rest is mirrored)
NCOL = NJC * JCHUNK  # 5120 columns per core
NSUB = JCHUNK // 512  # 2 matmul sub-tiles per chunk
GROUPS = ((0,), (1, 2, 3, 4))  # chunk groups per row-tile (PSUM holds 4)
ACT_CHUNKS = (0, 1)            # epilogue on ACT for these jc, DVE for rest
SQCOL = len(ACT_CHUNKS) * JCHUNK   # sq-fold matmul cols (ACT chunks only)
BCCOL = NCOL - SQCOL               # bc-epilogue cols (DVE chunks)

F16 = mybir.dt.float16
F32 = mybir.dt.float32


def _build_nc(loop_n=None, stage_bufs=4):
    nc = bacc.Bacc(None, target_bir_lowering=False)
    mt_d = nc.dram_tensor("mt", [D, NCOL], F16, kind="ExternalInput")
    sqf_d = nc.dram_tensor("sqf", [2, SQCOL], F16, kind="ExternalInput")
    bc_d = nc.dram_tensor("bc", [128, BCCOL], F16, kind="ExternalInput")
    bias_d = nc.dram_tensor("bias", [128, RT], F32, kind="ExternalInput")
    half_d = nc.dram_tensor("half", [128, RT], F32, kind="ExternalInput")
    out_d = nc.dram_tensor("out", [RPC, NCOL], F16, kind="ExternalOutput")

    with tile.TileContext(nc) as tc:
        with (
            tc.tile_pool(name="big", bufs=1) as big,
            tc.tile_pool(name="stage", bufs=stage_bufs) as stage_pool,
            tc.tile_pool(name="ps", bufs=4, space="PSUM") as psum,
        ):
            if loop_n is not None:
                with tc.For_i(0, loop_n, 1):
                    _emit_body(nc, tc, big, stage_pool, psum,
                               mt_d, sqf_d, bc_d, bias_d, half_d, out_d)
            else:
                _emit_body(nc, tc, big, stage_pool, psum,
                           mt_d, sqf_d, bc_d, bias_d, half_d, out_d)

    nc.compile()
    return nc


def _emit_body(nc, tc, big, stage_pool, psum,
               mt_d, sqf_d, bc_d, bias_d, half_d, out_d):
    mt0 = big.tile([128, NCOL], F16, tag="mt0")
    mt1 = big.tile([128, NCOL], F16, tag="mt1")
    sqp = big.tile([128, SQCOL], F16, tag="sqp")
    bc = big.tile([128, BCCOL], F16, tag="bc")
    w01 = big.tile([128, 128], F16, tag="w01")
    bias = big.tile([128, RT], F32, tag="bias")
    half = big.tile([128, RT], F32, tag="half")

    # first group (r=0, jc=0) only needs cols 0:1024; load those first
    # engine accesses must start at an aligned partition, so build the
    # zero-padded tiles by memsetting all 128 partitions first and then
    # DMA/memsetting the small top rows over them
    nc.sync.dma_start(mt0[:, 0:1024], mt_d[0:128, 0:1024])
    nc.sync.dma_start(mt1[:, 0:1024], mt_d[128:256, 0:1024])
    nc.vector.memset(sqp[:, 0:1024], 0.0)
    nc.sync.dma_start(sqp[0:2, 0:1024], sqf_d[:, 0:1024])
    nc.vector.memset(w01[:], 0.0)
    nc.vector.memset(w01[0:2, :], 1.0)
    nc.sync.dma_start(bias[:], bias_d[:])
    nc.sync.dma_start(half[:], half_d[:])
    nc.sync.dma_start(mt0[:, 1024:3072], mt_d[0:128, 1024:3072])
    nc.sync.dma_start(mt1[:, 1024:3072], mt_d[128:256, 1024:3072])
    nc.vector.memset(sqp[:, 1024:SQCOL], 0.0)
    nc.sync.dma_start(sqp[0:2, 1024:SQCOL], sqf_d[:, 1024:SQCOL])
    nc.sync.dma_start(bc[:], bc_d[:])
    nc.sync.dma_start(mt0[:, 3072:NCOL], mt_d[0:128, 3072:NCOL])
    nc.sync.dma_start(mt1[:, 3072:NCOL], mt_d[128:256, 3072:NCOL])

    for r in range(RT):
        lhs0 = mt0[:, r * 128:(r + 1) * 128]
        lhs1 = mt1[:, r * 128:(r + 1) * 128]
        for group in GROUPS:
            pss = [psum.tile([128, JCHUNK], F32, tag="ps", name=f"ps_{r}_{g}")
                   for g in group]
            # K-major sweeps: same stationary across all subtiles in the
            # group, then switch (ldweights changes are what cost time)
            for lhs, mt_src, st, sp, only_act in (
                (lhs0, mt0, True, False, False),
                (lhs1, mt1, False, True, None),   # stop unless w01 follows
                (w01[:], sqp, False, True, True),
            ):
                for gi, jc in enumerate(group):
                    act = jc in ACT_CHUNKS
                    if only_act is True and not act:
                        continue
                    stop = sp if only_act is not None else (not act)
                    for s in range(NSUB):
                        j0 = jc * JCHUNK + s * 512
                        o = pss[gi][:, s * 512:(s + 1) * 512]
                        nc.tensor.matmul(o, lhs, mt_src[:, j0:j0 + 512],
                                         start=st, stop=stop)
            for gi, jc in enumerate(group):
                ps = pss[gi]
                out_t = stage_pool.tile([128, JCHUNK], F16, tag="stage")
                if jc == 0:
                    # psum = gram - 0.5*sq_j can exceed 0.5*sq_i (making
                    # d2 = -2*psum + sq_i negative) only at i==j from fp
                    # rounding; clamp so the f16 output never sees it
                    dg = ps[:, r * 128:(r + 1) * 128]
                    nc.vector.tensor_scalar_min(dg, dg, half[:, r:r + 1])
                if jc in ACT_CHUNKS:
                    # ACT chunks emit d = sqrt(-2*psum + sq_i) directly
                    # (Sqrt supports the per-partition AP bias; Copy
                    # doesn't). DVE chunks emit d2; host sqrts those.
                    nc.scalar.activation(
                        out_t[:], ps[:],
                        mybir.ActivationFunctionType.Sqrt,
                        bias=bias[:, r:r + 1], scale=-2.0,
                    )
                else:
                    b0 = (jc * JCHUNK - SQCOL)
                    nc.vector.affine_then_add(
                        out_t[:], ps[:], bc[:, b0:b0 + JCHUNK],
                        scale=-2.0, bias=bias[:, r:r + 1],
                    )
                nc.sync.dma_start(
                    out_d[r * 128:(r + 1) * 128,
                          jc * JCHUNK:(jc + 1) * JCHUNK],
                    out_t[:],
                )


_NC_CACHE = None


def _get_nc():
    global _NC_CACHE
    if _NC_CACHE is None:
        _NC_CACHE = _build_nc()
    return _NC_CACHE


def _prep_core_inputs(xh, sq_all, c):
    nat = np.roll(xh, -c * RPC, axis=0)[:NCOL]      # [5120, 256] f16
    mtc = np.ascontiguousarray(nat.T)               # [256, 5120]
    sq = np.roll(sq_all, -c * RPC)[:NCOL]           # [5120] f32
    mh = (-0.5 * sq[:SQCOL]).astype(np.float32)
    hi = mh.astype(np.float16)
    lo = (mh - hi.astype(np.float32)).astype(np.float16)
    sqf = np.stack([hi, lo])                        # [2, SQCOL] f16
    bc = np.ascontiguousarray(
        np.broadcast_to(sq[SQCOL:].astype(np.float16), (128, BCCOL)))
    own = sq[:RPC].reshape(RT, 128).T               # [128, 8] f32
    return {"mt": mtc, "sqf": sqf, "bc": bc,
            "bias": np.ascontiguousarray(own),
            "half": np.ascontiguousarray(0.5 * own)}


def kernel(mapping: np.ndarray, **_kwargs) -> np.ndarray:
    mapping = np.asarray(mapping, dtype=np.float32)
    assert mapping.shape == (N, D)
    xh = mapping.astype(np.float16)
    sq_all = np.sum(xh.astype(np.float32) ** 2, axis=1)  # [N] f32

    in_maps = [_prep_core_inputs(xh, sq_all, c) for c in range(NCORES)]

    nc = _get_nc()
    res = run_bass_kernel_spmd(nc, in_maps, core_ids=list(range(NCORES)))

    out = np.empty((N, N), dtype=np.float32)
    for c in range(NCORES):
        blk = res.results[c]["out"].astype(np.float32)  # [1024, 5120]
        # ACT chunks (jc in ACT_CHUNKS) already hold d; DVE chunks hold d2
        for d in range(NJC):
            if d not in ACT_CHUNKS:
                sl = blk[:, d * RPC:(d + 1) * RPC]
                np.sqrt(np.maximum(sl, 0.0, out=sl), out=sl)
        r0 = c * RPC
        for d in range(NJC):
            j0 = ((c + d) % NCORES) * RPC
            out[r0:r0 + RPC, j0:j0 + RPC] = blk[:, d * RPC:(d + 1) * RPC]
    for i in range(NCORES):
        r0 = i * RPC
        for d in (5, 6, 7):
            j0 = ((i + d) % NCORES) * RPC
            out[r0:r0 + RPC, j0:j0 + RPC] = \
                out[j0:j0 + RPC, r0:r0 + RPC].T
    np.fill_diagonal(out, 0.0)
    return out


if __name__ == "__main__":
    rng = np.random.default_rng(0)
    x = rng.standard_normal((N, D)).astype(np.float32)
    o = kernel(mapping=x)
    print("out", o.shape, o.dtype, "sample", o[0, :4], "diag", np.abs(np.diag(o)).max())
